# revision 1
# baseline (speedup 1.0000x reference)
"""MixGCF negative-sampling + BPR loss kernel for 8x Trainium2 NeuronCores.

Strategy (data-parallel over batch, tables replicated):
  - 8 cores x 256 users each (2 chunks of 128 users = partitions).
  - Per chunk: indirect-DMA gather of user/pos rows and all 128 candidate
    item rows (128 rows per call, 1KB rows), DVE mul+reduce for per-hop
    scores, argmax via max/iota-onehot trick, second small indirect gather
    of the selected rows, then the BPR loss reduced to per-partition
    partial sums. Host combines 8x[128,2] partials into 3 scalars.
"""
import sys

sys.path.insert(0, "/opt/trn_rl_repo")
import numpy as np

N_USERS = 200000
N_ITEMS = 200000
HOPS = 4
DIM = 64
BATCH = 2048
N_NEGS = 64
K = 2
DECAY = 1e-4
NCORES = 8
ROW = HOPS * DIM          # 256 f32 per table row
B_LOC = BATCH // NCORES   # 256 users per core
NCHUNK = B_LOC // 128     # 2 chunks of 128 users
NCAND = K * N_NEGS        # 128 candidates per user
SUB = 32                  # candidates gathered/scored per inner step
NSUB = NCAND // SUB       # 4

_CACHE = {}


def _build_bass(stage=99):
    import concourse.bass as bass
    import concourse.tile as tile
    from concourse import bacc, mybir

    f32 = mybir.dt.float32
    i32 = mybir.dt.int32
    Alu = mybir.AluOpType
    Act = mybir.ActivationFunctionType

    nc = bacc.Bacc("TRN2", target_bir_lowering=False, debug=False,
                   num_devices=NCORES)
    user_tab = nc.dram_tensor("user_tab", [N_USERS, ROW], f32,
                              kind="ExternalInput").ap()
    item_tab = nc.dram_tensor("item_tab", [N_ITEMS, ROW], f32,
                              kind="ExternalInput").ap()
    uid = nc.dram_tensor("uid", [128, NCHUNK], i32, kind="ExternalInput").ap()
    pid = nc.dram_tensor("pid", [128, NCHUNK], i32, kind="ExternalInput").ap()
    nid = nc.dram_tensor("nid", [128, NCHUNK, NCAND], i32,
                         kind="ExternalInput").ap()
    seed = nc.dram_tensor("seed", [128, NCHUNK], f32,
                          kind="ExternalInput").ap()
    part = nc.dram_tensor("part", [128, 2], f32, kind="ExternalOutput").ap()

    with tile.TileContext(nc) as tc:
        with tc.tile_pool(name="meta", bufs=1) as meta, \
             tc.tile_pool(name="gat", bufs=5) as gatp, \
             tc.tile_pool(name="sp", bufs=2) as spp, \
             tc.tile_pool(name="sn", bufs=2) as snp, \
             tc.tile_pool(name="sel", bufs=2) as selp, \
             tc.tile_pool(name="small", bufs=2) as smallp:
            # ---- static/meta staging ----
            uid_t = meta.tile([128, NCHUNK], i32)
            pid_t = meta.tile([128, NCHUNK], i32)
            nid_t = meta.tile([128, NCHUNK, NCAND], i32)
            seed_t = meta.tile([128, NCHUNK], f32)
            nc.sync.dma_start(uid_t[:], uid)
            nc.sync.dma_start(pid_t[:], pid)
            nc.sync.dma_start(nid_t[:], nid)
            nc.sync.dma_start(seed_t[:], seed)

            oms_t = meta.tile([128, NCHUNK], f32)   # 1 - seed
            nc.vector.tensor_scalar(oms_t[:], seed_t[:], -1.0, 1.0,
                                    Alu.mult, Alu.add)
            nidf_t = meta.tile([128, NCHUNK, NCAND], f32)
            nc.vector.tensor_copy(nidf_t[:], nid_t[:])

            iota_rev = meta.tile([128, N_NEGS], i32)  # 64 - n
            nc.gpsimd.iota(iota_rev[:], pattern=[[-1, N_NEGS]], base=N_NEGS,
                           channel_multiplier=0)
            iota_rev_f = meta.tile([128, N_NEGS], f32)
            nc.vector.tensor_copy(iota_rev_f[:], iota_rev[:])

            part_t = meta.tile([128, 2], f32)
            nc.vector.memset(part_t[:], 0.0)

            for ch in range(NCHUNK if stage >= 2 else 0):
                seed_ap = seed_t[:, ch:ch + 1]
                oms_ap = oms_t[:, ch:ch + 1]

                # ---- user/pos row gathers ----
                s_t = spp.tile([128, ROW], f32, tag="s")
                p_t = spp.tile([128, ROW], f32, tag="p")
                nc.gpsimd.indirect_dma_start(
                    out=s_t[:], out_offset=None, in_=user_tab,
                    in_offset=bass.IndirectOffsetOnAxis(
                        ap=uid_t[:, ch:ch + 1], axis=0))
                nc.gpsimd.indirect_dma_start(
                    out=p_t[:], out_offset=None, in_=item_tab,
                    in_offset=bass.IndirectOffsetOnAxis(
                        ap=pid_t[:, ch:ch + 1], axis=0))

                if stage <= 2:
                    continue
                # ---- candidate gathers + scoring ----
                sn_t = snp.tile([128, NCAND, HOPS], f32, tag="sn")
                for nt in range(NSUB):
                    gat = gatp.tile([128, SUB, ROW], f32, tag="gat")
                    for j in range(SUB):
                        col = nt * SUB + j
                        nc.gpsimd.indirect_dma_start(
                            out=gat[:, j], out_offset=None,
                            in_=item_tab,
                            in_offset=bass.IndirectOffsetOnAxis(
                                ap=nid_t[:, ch, col:col + 1], axis=0))
                    nc.vector.tensor_tensor(
                        out=gat[:], in0=gat[:],
                        in1=s_t[:].unsqueeze(1).to_broadcast([128, SUB, ROW]),
                        op=Alu.mult)
                    nc.vector.tensor_reduce(
                        out=sn_t[:, nt * SUB:(nt + 1) * SUB, :],
                        in_=gat[:].rearrange("p s (h d) -> p s h d", h=HOPS),
                        axis=mybir.AxisListType.X, op=Alu.add)

                if stage <= 3:
                    continue
                # ---- argmax per (k, hop) via onehot trick ----
                g_t = snp.tile([128, NCAND, HOPS], f32, tag="g")
                nc.vector.tensor_scalar_mul(g_t[:], sn_t[:], oms_ap)

                candf = smallp.tile([128, K * HOPS], f32, tag="candf")
                for k in range(K):
                    gk = g_t[:, k * N_NEGS:(k + 1) * N_NEGS, :].transpose(
                        [0, 2, 1])                     # [128, H, N]
                    m_k = smallp.tile([128, HOPS], f32, tag="mk")
                    nc.vector.tensor_reduce(out=m_k[:], in_=gk,
                                            axis=mybir.AxisListType.X,
                                            op=Alu.max)
                    eq = smallp.tile([128, HOPS, N_NEGS], f32, tag="eq")
                    nc.vector.tensor_tensor(
                        out=eq[:], in0=gk,
                        in1=m_k[:].unsqueeze(2).to_broadcast(
                            [128, HOPS, N_NEGS]),
                        op=Alu.is_equal)
                    w = smallp.tile([128, HOPS, N_NEGS], f32, tag="w")
                    nc.vector.tensor_tensor(
                        out=w[:], in0=eq[:],
                        in1=iota_rev_f[:].unsqueeze(1).to_broadcast(
                            [128, HOPS, N_NEGS]),
                        op=Alu.mult)
                    wmax = smallp.tile([128, HOPS], f32, tag="wmax")
                    nc.vector.tensor_reduce(out=wmax[:], in_=w[:],
                                            axis=mybir.AxisListType.X,
                                            op=Alu.max)
                    onehot = smallp.tile([128, HOPS, N_NEGS], f32, tag="oh")
                    nc.vector.tensor_tensor(
                        out=onehot[:],
                        in0=iota_rev_f[:].unsqueeze(1).to_broadcast(
                            [128, HOPS, N_NEGS]),
                        in1=wmax[:].unsqueeze(2).to_broadcast(
                            [128, HOPS, N_NEGS]),
                        op=Alu.is_equal)
                    idsel = smallp.tile([128, HOPS, N_NEGS], f32, tag="ids")
                    nc.vector.tensor_tensor(
                        out=idsel[:], in0=onehot[:],
                        in1=nidf_t[:, ch, k * N_NEGS:(k + 1) * N_NEGS]
                            .unsqueeze(1).to_broadcast([128, HOPS, N_NEGS]),
                        op=Alu.mult)
                    nc.vector.tensor_reduce(out=candf[:, k * HOPS:(k + 1) * HOPS], in_=idsel[:],
                                            axis=mybir.AxisListType.X,
                                            op=Alu.add)

                cand_i = smallp.tile([128, K * HOPS], i32, tag="candi")
                nc.vector.tensor_copy(cand_i[:], candf[:])

                if stage <= 4:
                    continue
                # ---- gather selected rows ----
                selr = selp.tile([128, K * HOPS, ROW], f32, tag="selr")
                for j in range(K * HOPS):
                    nc.gpsimd.indirect_dma_start(
                        out=selr[:, j], out_offset=None,
                        in_=item_tab,
                        in_offset=bass.IndirectOffsetOnAxis(
                            ap=cand_i[:, j:j + 1], axis=0))

                if stage <= 5:
                    continue
                # ---- hop sums ----
                u_sum = smallp.tile([128, DIM], f32, tag="usum")
                p_sum = smallp.tile([128, DIM], f32, tag="psum")
                nc.vector.tensor_reduce(
                    out=u_sum[:],
                    in_=s_t[:].rearrange("p (h d) -> p h d",
                                         h=HOPS).transpose([0, 2, 1]),
                    axis=mybir.AxisListType.X, op=Alu.add)
                nc.vector.tensor_reduce(
                    out=p_sum[:],
                    in_=p_t[:].rearrange("p (h d) -> p h d",
                                         h=HOPS).transpose([0, 2, 1]),
                    axis=mybir.AxisListType.X, op=Alu.add)

                psum_seed = smallp.tile([128, DIM], f32, tag="pseed")
                nc.vector.tensor_scalar_mul(psum_seed[:], p_sum[:], seed_ap)

                # n_sum_k = oms * sum_h selr[k,h-diag] + seed * p_sum
                n_sums = []
                for k in range(K):
                    r_k = smallp.tile([128, DIM], f32, tag=f"rk{k}")
                    nc.vector.tensor_add(
                        r_k[:], selr[:, 4 * k + 0, 0:DIM],
                        selr[:, 4 * k + 1, DIM:2 * DIM])
                    nc.vector.tensor_add(r_k[:], r_k[:],
                                         selr[:, 4 * k + 2, 2 * DIM:3 * DIM])
                    nc.vector.tensor_add(r_k[:], r_k[:],
                                         selr[:, 4 * k + 3, 3 * DIM:4 * DIM])
                    n_k = smallp.tile([128, DIM], f32, tag=f"nk{k}")
                    nc.vector.tensor_scalar_mul(n_k[:], r_k[:], oms_ap)
                    nc.vector.tensor_add(n_k[:], n_k[:], psum_seed[:])
                    n_sums.append(n_k)

                if stage <= 6:
                    continue
                # ---- loss pieces (scale 1/16 folds the /HOPS means) ----
                S = 1.0 / (HOPS * HOPS)
                tmp = smallp.tile([128, DIM], f32, tag="tmp")

                def dotp(out_ap, a, b):
                    nc.vector.tensor_tensor(out=tmp[:], in0=a, in1=b,
                                            op=Alu.mult)
                    nc.vector.tensor_reduce(out=out_ap, in_=tmp[:],
                                            axis=mybir.AxisListType.X,
                                            op=Alu.add)

                pos_s = smallp.tile([128, 1], f32, tag="poss")   # raw (x16)
                dotp(pos_s[:], u_sum[:], p_sum[:])
                neg_s = []
                for k in range(K):
                    ns = smallp.tile([128, 1], f32, tag=f"negs{k}")
                    dotp(ns[:], u_sum[:], n_sums[k][:])
                    neg_s.append(ns)

                sq = smallp.tile([128, 4], f32, tag="sq")        # raw (x16)
                dotp(sq[:, 0:1], u_sum[:], u_sum[:])
                dotp(sq[:, 1:2], p_sum[:], p_sum[:])
                dotp(sq[:, 2:3], n_sums[0][:], n_sums[0][:])
                dotp(sq[:, 3:4], n_sums[1][:], n_sums[1][:])
                sq_tot = smallp.tile([128, 1], f32, tag="sqtot")
                nc.vector.tensor_add(sq_tot[:], sq[:, 0:1], sq[:, 1:2])
                nc.vector.tensor_add(sq_tot[:], sq_tot[:], sq[:, 2:3])
                nc.vector.tensor_add(sq_tot[:], sq_tot[:], sq[:, 3:4])
                nc.vector.tensor_scalar_mul(sq_tot[:], sq_tot[:], S)

                negpos = smallp.tile([128, 1], f32, tag="negpos")
                nc.vector.tensor_scalar_mul(negpos[:], pos_s[:], -S)
                e01 = smallp.tile([128, 2], f32, tag="e01")
                for k in range(K):
                    nc.scalar.activation(out=e01[:, k:k + 1],
                                         in_=neg_s[k][:], func=Act.Exp,
                                         bias=negpos[:], scale=S)
                if stage <= 8:
                    continue
                esum = smallp.tile([128, 1], f32, tag="esum")
                nc.vector.tensor_add(esum[:], e01[:, 0:1], e01[:, 1:2])
                mf = smallp.tile([128, 1], f32, tag="mf")
                nc.scalar.activation(out=mf[:], in_=esum[:], func=Act.Ln,
                                     bias=1.0, scale=1.0)

                nc.vector.tensor_add(part_t[:, 0:1], part_t[:, 0:1], mf[:])
                nc.vector.tensor_add(part_t[:, 1:2], part_t[:, 1:2],
                                     sq_tot[:])

            nc.sync.dma_start(part, part_t[:])
    nc.compile()
    return nc


def _build_runner(nc):
    import jax
    from jax.sharding import Mesh, PartitionSpec
    from jax.experimental.shard_map import shard_map
    from concourse import mybir
    from concourse.bass2jax import (install_neuronx_cc_hook,
                                    partition_id_tensor, _bass_exec_p)

    install_neuronx_cc_hook()
    partition_name = (nc.partition_id_tensor.name
                      if nc.partition_id_tensor else None)
    REPLICATED = {"user_tab", "item_tab"}

    in_names, out_names, out_avals, zero_outs = [], [], [], []
    for alloc in nc.m.functions[0].allocations:
        if not isinstance(alloc, mybir.MemoryLocationSet):
            continue
        name = alloc.memorylocations[0].name
        if alloc.kind == "ExternalInput":
            if name != partition_name:
                in_names.append(name)
        elif alloc.kind == "ExternalOutput":
            out_names.append(name)
            shape = tuple(alloc.tensor_shape)
            dtype = mybir.dt.np(alloc.dtype)
            out_avals.append(jax.core.ShapedArray(shape, dtype))
            zero_outs.append(np.zeros(shape, dtype))
    n_params = len(in_names)
    n_outs = len(out_avals)
    all_in_names = list(in_names) + list(out_names)
    if partition_name is not None:
        all_in_names.append(partition_name)

    def _body(*args):
        operands = list(args)
        if partition_name is not None:
            operands.append(partition_id_tensor())
        outs = _bass_exec_p.bind(
            *operands, out_avals=tuple(out_avals),
            in_names=tuple(all_in_names), out_names=tuple(out_names),
            lowering_input_output_aliases=(), sim_require_finite=True,
            sim_require_nnan=True, nc=nc)
        return tuple(outs)

    devices = jax.devices()[:NCORES]
    mesh = Mesh(np.asarray(devices), ("core",))
    spec_of = [
        PartitionSpec() if name in REPLICATED else PartitionSpec("core")
        for name in in_names
    ]
    in_specs = tuple(spec_of) + (PartitionSpec("core"),) * n_outs
    out_specs = (PartitionSpec("core"),) * n_outs
    sharded = jax.jit(
        shard_map(_body, mesh=mesh, in_specs=in_specs, out_specs=out_specs,
                  check_rep=False),
        keep_unused=True)
    shard_s = jax.sharding.NamedSharding(mesh, PartitionSpec("core"))
    repl_s = jax.sharding.NamedSharding(mesh, PartitionSpec())

    def run(per_core_maps, replicated_map):
        args = []
        for i, name in enumerate(in_names):
            if name in REPLICATED:
                args.append(jax.device_put(replicated_map[name], repl_s))
            else:
                args.append(jax.device_put(
                    np.concatenate([m[name] for m in per_core_maps], axis=0),
                    shard_s))
        for z in zero_outs:
            args.append(jax.device_put(
                np.zeros((NCORES * z.shape[0], *z.shape[1:]), z.dtype),
                shard_s))
        outs = sharded(*args)
        return [
            {name: np.asarray(outs[i]).reshape(NCORES, *out_avals[i].shape)[c]
             for i, name in enumerate(out_names)}
            for c in range(NCORES)
        ]

    return run


def _get_runner():
    import os
    if "run" not in _CACHE:
        nc = _build_bass(int(os.environ.get("KSTAGE", "99")))
        _CACHE["nc"] = nc
        _CACHE["run"] = _build_runner(nc)
    return _CACHE["run"]


def make_in_maps(user_gcn_emb, item_gcn_emb, seed_embed, user, pos_item,
                 neg_item):
    """Host-side sharding/marshalling into per-core input maps."""
    user = np.asarray(user).astype(np.int32)
    pos_item = np.asarray(pos_item).astype(np.int32)
    neg_item = np.asarray(neg_item).astype(np.int32)
    seed = np.asarray(seed_embed, dtype=np.float32).reshape(BATCH)
    per_core = []
    for c in range(NCORES):
        lo = c * B_LOC
        # partition-major: [128 partitions, NCHUNK]
        u = user[lo:lo + B_LOC].reshape(NCHUNK, 128).T.copy()
        p = pos_item[lo:lo + B_LOC].reshape(NCHUNK, 128).T.copy()
        n = (neg_item[lo:lo + B_LOC]
             .reshape(NCHUNK, 128, NCAND).transpose(1, 0, 2).copy())
        s = seed[lo:lo + B_LOC].reshape(NCHUNK, 128).T.copy()
        per_core.append({"uid": u, "pid": p, "nid": n, "seed": s})
    replicated = {
        "user_tab": np.ascontiguousarray(
            np.asarray(user_gcn_emb, dtype=np.float32).reshape(N_USERS, ROW)),
        "item_tab": np.ascontiguousarray(
            np.asarray(item_gcn_emb, dtype=np.float32).reshape(N_ITEMS, ROW)),
    }
    return per_core, replicated


def combine(results):
    mf_sum = 0.0
    sq_sum = 0.0
    for r in results:
        mf_sum += float(r["part"][:, 0].astype(np.float64).sum())
        sq_sum += float(r["part"][:, 1].astype(np.float64).sum())
    mf_loss = np.float32(mf_sum / BATCH)
    emb_loss = np.float32(DECAY * sq_sum / 2.0 / BATCH)
    loss = np.float32(mf_loss + emb_loss)
    return loss, mf_loss, emb_loss


def kernel(user_gcn_emb, item_gcn_emb, seed_embed, user, pos_item, neg_item):
    run = _get_runner()
    per_core, replicated = make_in_maps(user_gcn_emb, item_gcn_emb,
                                        seed_embed, user, pos_item, neg_item)
    results = run(per_core, replicated)
    return combine(results)



# revision 9
# speedup vs baseline: 1.0674x; 1.0674x over previous
"""MixGCF negative-sampling + BPR loss kernel for 8x Trainium2 NeuronCores.

Strategy (data-parallel over batch, tables replicated):
  - 8 cores x 256 users each (2 chunks of 128 users = partitions).
  - Candidate rows are fetched with dma_gather (thousands of rows per call)
    from a host-provided bf16 copy of the item table. dma_gather indices
    are int16, so the 200k-row table is covered by 13 overlapping 32k-row
    windows; the host assigns each candidate to a window (chain DP per
    user) so that rows land on their user's partition, padding short
    cells with window-base rows that are masked out of the argmax.
  - Scoring (s.n per hop) in bf16: elementwise mult split between the
    gpsimd and vector engines, hop-reduce + masked argmax on vector.
  - Selected rows are re-fetched in f32 with per-row indirect DMAs and
    fed into the BPR loss, reduced to per-partition partial sums. Host
    combines 8x[128,2] partials into the 3 scalars.
"""
import sys

sys.path.insert(0, "/opt/trn_rl_repo")
import numpy as np
import ml_dtypes

N_USERS = 200000
N_ITEMS = 200000
HOPS = 4
DIM = 64
BATCH = 2048
N_NEGS = 64
K = 2
DECAY = 1e-4
NCORES = 8
ROW = HOPS * DIM          # 256 f32 per table row
B_LOC = BATCH // NCORES   # 256 users per core
NCHUNK = B_LOC // 128     # 2 chunks of 128 users = partitions
NCAND = K * N_NEGS        # 128 candidates per user
NWIN = 13                 # overlapping int16-addressable windows
WSTRIDE = 16384           # window stride; width is 2*WSTRIDE = 32768
NEG_BIG = -1.0e30

_CACHE = {}


# ---------------------------------------------------------------------------
# Host-side window assignment
# ---------------------------------------------------------------------------

def _cell_assign(draws):
    """Assign each of a user's draws to a window via minimal-cap chain DP.

    Windows are [w*WSTRIDE, w*WSTRIDE + 2*WSTRIDE); a draw in segment
    s = id // WSTRIDE may go to window s or s-1.  Returns per-draw window.
    """
    seg = draws // WSTRIDE
    n = np.bincount(seg, minlength=NWIN)

    def feasible(cap):
        k_next = 0
        for s in range(NWIN - 1, 0, -1):
            k = n[s] + k_next - cap
            if k < 0:
                k = 0
            elif k > n[s]:
                return False
            k_next = k
        return n[0] + k_next <= cap

    lo, hi = -(-len(draws) // NWIN), int(n.max())
    while lo < hi:
        mid = (lo + hi) // 2
        if feasible(mid):
            hi = mid
        else:
            lo = mid + 1
    cap = lo
    kv = np.zeros(NWIN + 1, dtype=np.int64)
    for s in range(NWIN - 1, 0, -1):
        kv[s] = max(0, n[s] + kv[s + 1] - cap)
    win = seg.copy()
    for s in range(1, NWIN):
        if kv[s]:
            win[np.where(seg == s)[0][:kv[s]]] = s - 1
    return win


def _wrap16(vals):
    """int16 list [n] -> dma_gather index layout [128, n//16*8->n/16] tile."""
    n = vals.shape[0]
    a = vals.reshape(n // 16, 16).T.astype(np.int16)   # [16, n//16]
    return np.tile(a, (8, 1))


def _plan(neg_item):
    """Window plan shared by all cores: slot counts per (chunk, window)."""
    neg4 = neg_item.reshape(NCORES, NCHUNK, 128, NCAND)
    wins = np.zeros(neg4.shape, dtype=np.int8)
    slots = np.zeros((NCHUNK, NWIN), dtype=np.int64)
    for c in range(NCORES):
        for ch in range(NCHUNK):
            for p in range(128):
                w = _cell_assign(neg4[c, ch, p].astype(np.int64))
                wins[c, ch, p] = w
                cnt = np.bincount(w, minlength=NWIN)
                slots[ch] = np.maximum(slots[ch], cnt)
    return wins, slots


def make_in_maps(user_gcn_emb, item_gcn_emb, seed_embed, user, pos_item,
                 neg_item):
    """Host-side sharding/marshalling into per-core input maps."""
    user = np.asarray(user).astype(np.int32)
    pos = np.asarray(pos_item).astype(np.int32)
    neg = np.asarray(neg_item).astype(np.int64)
    seed = np.asarray(seed_embed, dtype=np.float32).reshape(BATCH)

    wins, slots = _plan(neg)
    offs = [np.concatenate([[0], np.cumsum(slots[ch])]) for ch in range(NCHUNK)]
    S = [int(offs[ch][-1]) for ch in range(NCHUNK)]  # positions per chunk

    neg4 = neg.reshape(NCORES, NCHUNK, 128, NCAND)
    per_core = []
    for c in range(NCORES):
        lo = c * B_LOC
        u = user[lo:lo + B_LOC].reshape(NCHUNK, 128).T.copy()
        p = pos[lo:lo + B_LOC].reshape(NCHUNK, 128).T.copy()
        s = seed[lo:lo + B_LOC].reshape(NCHUNK, 128).T.copy()

        widx_blocks = []
        mneg = [np.full((128, K, S[ch]), NEG_BIG, np.float32)
                for ch in range(NCHUNK)]
        iota = [np.zeros((128, S[ch]), np.float32) for ch in range(NCHUNK)]
        nidp = [np.zeros((128, S[ch]), np.float32) for ch in range(NCHUNK)]
        for ch in range(NCHUNK):
            for w in range(NWIN):
                sl = int(slots[ch][w])
                if sl == 0:
                    continue
                base = w * WSTRIDE
                idx16 = np.zeros((sl * 128,), np.int64)  # pad idx -> 0
                off = int(offs[ch][w])
                for pa in range(128):
                    ids = neg4[c, ch, pa]
                    js = np.where(wins[c, ch, pa] == w)[0]
                    for j, draw in enumerate(js):
                        idx16[j * 128 + pa] = ids[draw] - base
                        pos_col = off + j
                        k, n = draw // N_NEGS, draw % N_NEGS
                        mneg[ch][pa, k, pos_col] = 0.0
                        iota[ch][pa, pos_col] = N_NEGS - n
                        nidp[ch][pa, pos_col] = ids[draw]
                widx_blocks.append(_wrap16(idx16))
        widx = np.concatenate(widx_blocks, axis=1)
        aux = np.concatenate(
            [np.concatenate([mneg[ch].reshape(128, K * S[ch]),
                             iota[ch], nidp[ch]], axis=1)
             for ch in range(NCHUNK)], axis=1).astype(np.float32)
        per_core.append({"uid": u, "pid": p, "seed": s,
                         "widx": widx, "aux": aux})

    replicated = {
        "user_tab": np.ascontiguousarray(
            np.asarray(user_gcn_emb, dtype=np.float32).reshape(N_USERS, ROW)),
        "item_tab": np.ascontiguousarray(
            np.asarray(item_gcn_emb, dtype=np.float32).reshape(N_ITEMS, ROW)),
    }
    replicated["item16"] = replicated["item_tab"].astype(ml_dtypes.bfloat16)
    return per_core, replicated, slots


# ---------------------------------------------------------------------------
# Device kernel
# ---------------------------------------------------------------------------

def _build_bass(slots):
    import concourse.bass as bass
    import concourse.tile as tile
    from concourse import bacc, mybir

    f32 = mybir.dt.float32
    bf16 = mybir.dt.bfloat16
    i32 = mybir.dt.int32
    i16 = mybir.dt.int16
    Alu = mybir.AluOpType
    Act = mybir.ActivationFunctionType

    offs = [np.concatenate([[0], np.cumsum(slots[ch])]) for ch in range(NCHUNK)]
    S = [int(offs[ch][-1]) for ch in range(NCHUNK)]
    SMAX = int(max(slots[0].max(), slots[1].max()))
    TW = 8 * (S[0] + S[1])          # widx columns
    AW = (K + 2) * (S[0] + S[1])    # aux columns

    nc = bacc.Bacc("TRN2", target_bir_lowering=False, debug=False,
                   num_devices=NCORES)
    user_tab = nc.dram_tensor("user_tab", [N_USERS, ROW], f32,
                              kind="ExternalInput").ap()
    item_tab = nc.dram_tensor("item_tab", [N_ITEMS, ROW], f32,
                              kind="ExternalInput").ap()
    item16 = nc.dram_tensor("item16", [N_ITEMS, ROW], bf16,
                            kind="ExternalInput").ap()
    uid = nc.dram_tensor("uid", [128, NCHUNK], i32, kind="ExternalInput").ap()
    pid = nc.dram_tensor("pid", [128, NCHUNK], i32, kind="ExternalInput").ap()
    seed = nc.dram_tensor("seed", [128, NCHUNK], f32,
                          kind="ExternalInput").ap()
    widx = nc.dram_tensor("widx", [128, TW], i16, kind="ExternalInput").ap()
    aux = nc.dram_tensor("aux", [128, AW], f32, kind="ExternalInput").ap()
    part = nc.dram_tensor("part", [128, 2], f32, kind="ExternalOutput").ap()

    with tile.TileContext(nc) as tc:
        with tc.tile_pool(name="meta", bufs=1) as meta, \
             tc.tile_pool(name="gat", bufs=4) as gatp, \
             tc.tile_pool(name="sel", bufs=2) as selp, \
             tc.tile_pool(name="small", bufs=2) as smallp:
            # ---- static/meta staging ----
            uid_t = meta.tile([128, NCHUNK], i32)
            pid_t = meta.tile([128, NCHUNK], i32)
            seed_t = meta.tile([128, NCHUNK], f32)
            widx_t = meta.tile([128, TW], i16)
            aux_t = meta.tile([128, AW], f32)
            nc.sync.dma_start(uid_t[:], uid)
            nc.sync.dma_start(pid_t[:], pid)
            nc.sync.dma_start(seed_t[:], seed)
            nc.sync.dma_start(widx_t[:], widx)
            nc.sync.dma_start(aux_t[:], aux)

            # aux layout per chunk: [mneg(K*S), iota(S), nid(S)]
            aux_off = [0, (K + 2) * S[0]]

            def mneg_ap(ch, k):
                o = aux_off[ch] + k * S[ch]
                return aux_t[:, o:o + S[ch]]

            def iota_ap(ch):
                o = aux_off[ch] + K * S[ch]
                return aux_t[:, o:o + S[ch]]

            def nid_ap(ch):
                o = aux_off[ch] + (K + 1) * S[ch]
                return aux_t[:, o:o + S[ch]]

            oms_t = meta.tile([128, NCHUNK], f32)   # 1 - seed
            nc.vector.tensor_scalar(oms_t[:], seed_t[:], -1.0, 1.0,
                                    Alu.mult, Alu.add)

            part_t = meta.tile([128, 2], f32)
            nc.vector.memset(part_t[:], 0.0)

            # ---- user/pos rows (one [128,1] indirect call per chunk) ----
            s2_t = meta.tile([128, NCHUNK, ROW], f32)
            p2_t = meta.tile([128, NCHUNK, ROW], f32)
            for ch in range(NCHUNK):
                nc.gpsimd.indirect_dma_start(
                    out=s2_t[:, ch], out_offset=None, in_=user_tab,
                    in_offset=bass.IndirectOffsetOnAxis(
                        ap=uid_t[:, ch:ch + 1], axis=0))
                nc.gpsimd.indirect_dma_start(
                    out=p2_t[:, ch], out_offset=None, in_=item_tab,
                    in_offset=bass.IndirectOffsetOnAxis(
                        ap=pid_t[:, ch:ch + 1], axis=0))
            s16_t = meta.tile([128, NCHUNK, ROW], bf16)
            nc.vector.tensor_copy(s16_t[:], s2_t[:])

            # ---- windowed candidate gathers + scoring ----
            sn = [meta.tile([128, S[ch], HOPS], f32, name=f"sn{ch}")
                  for ch in range(NCHUNK)]
            calls = [(ch, w) for ch in range(NCHUNK) for w in range(NWIN)
                     if slots[ch][w] > 0]
            woff = 0
            for ci, (ch, w) in enumerate(calls):
                sl = int(slots[ch][w])
                base = w * WSTRIDE
                hi = min(base + 2 * WSTRIDE, N_ITEMS)
                g = gatp.tile([128, SMAX, ROW], bf16, tag="gat")
                nc.gpsimd.dma_gather(
                    out_ap=g[:, :sl, :], in_ap=item16[base:hi],
                    idxs_ap=widx_t[:, woff:woff + sl * 8],
                    num_idxs=sl * 128, num_idxs_reg=sl * 128,
                    elem_size=ROW, single_packet=False)
                woff += sl * 8
                # elementwise s.n product; alternate engines to split load
                eng = nc.gpsimd if ci % 2 == 0 else nc.vector
                eng.tensor_tensor(
                    out=g[:, :sl, :], in0=g[:, :sl, :],
                    in1=s16_t[:, ch].unsqueeze(1).to_broadcast(
                        [128, sl, ROW]),
                    op=Alu.mult)
                off = int(offs[ch][w])
                nc.vector.tensor_reduce(
                    out=sn[ch][:, off:off + sl, :],
                    in_=g[:, :sl, :].rearrange("p s (h d) -> p s h d",
                                               h=HOPS),
                    axis=mybir.AxisListType.X, op=Alu.add)

            # ---- masked argmax per (chunk, k) ----
            candf = meta.tile([128, NCHUNK * K * HOPS], f32)
            for ch in range(NCHUNK):
                oms_ap = oms_t[:, ch:ch + 1]
                tmp = smallp.tile([128, S[ch], HOPS], f32, tag=f"tmp{ch}")
                nc.vector.tensor_scalar_mul(tmp[:], sn[ch][:], oms_ap)
                for k in range(K):
                    tg = smallp.tile([128, S[ch], HOPS], f32, tag=f"tg{ch}")
                    nc.vector.tensor_tensor(
                        out=tg[:], in0=tmp[:],
                        in1=mneg_ap(ch, k).unsqueeze(2).to_broadcast(
                            [128, S[ch], HOPS]),
                        op=Alu.add)
                    gk = tg[:].transpose([0, 2, 1])      # [128, H, S]
                    m_k = smallp.tile([128, HOPS], f32, tag="mk")
                    nc.vector.tensor_reduce(out=m_k[:], in_=gk,
                                            axis=mybir.AxisListType.X,
                                            op=Alu.max)
                    eq = smallp.tile([128, HOPS, S[ch]], f32, tag=f"eq{ch}")
                    nc.vector.tensor_tensor(
                        out=eq[:], in0=gk,
                        in1=m_k[:].unsqueeze(2).to_broadcast(
                            [128, HOPS, S[ch]]),
                        op=Alu.is_equal)
                    w_t = smallp.tile([128, HOPS, S[ch]], f32, tag=f"w{ch}")
                    nc.vector.tensor_tensor(
                        out=w_t[:], in0=eq[:],
                        in1=iota_ap(ch).unsqueeze(1).to_broadcast(
                            [128, HOPS, S[ch]]),
                        op=Alu.mult)
                    wmax = smallp.tile([128, HOPS], f32, tag="wmax")
                    nc.vector.tensor_reduce(out=wmax[:], in_=w_t[:],
                                            axis=mybir.AxisListType.X,
                                            op=Alu.max)
                    onehot = smallp.tile([128, HOPS, S[ch]], f32,
                                         tag=f"oh{ch}")
                    nc.vector.tensor_tensor(
                        out=onehot[:], in0=w_t[:],
                        in1=wmax[:].unsqueeze(2).to_broadcast(
                            [128, HOPS, S[ch]]),
                        op=Alu.is_equal)
                    idsel = smallp.tile([128, HOPS, S[ch]], f32,
                                        tag=f"ids{ch}")
                    nc.vector.tensor_tensor(
                        out=idsel[:], in0=onehot[:],
                        in1=nid_ap(ch).unsqueeze(1).to_broadcast(
                            [128, HOPS, S[ch]]),
                        op=Alu.mult)
                    col = (ch * K + k) * HOPS
                    nc.vector.tensor_reduce(
                        out=candf[:, col:col + HOPS], in_=idsel[:],
                        axis=mybir.AxisListType.X, op=Alu.add)

            cand_i = meta.tile([128, NCHUNK * K * HOPS], i32)
            nc.vector.tensor_copy(cand_i[:], candf[:])

            # ---- gather selected rows (f32, [128,1] per row) ----
            selr = meta.tile([128, NCHUNK * K * HOPS, ROW], f32)
            for j in range(NCHUNK * K * HOPS):
                nc.gpsimd.indirect_dma_start(
                    out=selr[:, j], out_offset=None, in_=item_tab,
                    in_offset=bass.IndirectOffsetOnAxis(
                        ap=cand_i[:, j:j + 1], axis=0))

            # ---- loss pieces per chunk (scale 1/16 folds the /HOPS) ----
            Ssc = 1.0 / (HOPS * HOPS)
            for ch in range(NCHUNK):
                seed_ap = seed_t[:, ch:ch + 1]
                oms_ap = oms_t[:, ch:ch + 1]
                u_sum = smallp.tile([128, DIM], f32, tag="usum")
                p_sum = smallp.tile([128, DIM], f32, tag="psum")
                nc.vector.tensor_reduce(
                    out=u_sum[:],
                    in_=s2_t[:, ch].rearrange("p (h d) -> p h d",
                                              h=HOPS).transpose([0, 2, 1]),
                    axis=mybir.AxisListType.X, op=Alu.add)
                nc.vector.tensor_reduce(
                    out=p_sum[:],
                    in_=p2_t[:, ch].rearrange("p (h d) -> p h d",
                                              h=HOPS).transpose([0, 2, 1]),
                    axis=mybir.AxisListType.X, op=Alu.add)

                psum_seed = smallp.tile([128, DIM], f32, tag="pseed")
                nc.vector.tensor_scalar_mul(psum_seed[:], p_sum[:], seed_ap)

                n_sums = []
                for k in range(K):
                    base_j = ch * K * HOPS + k * HOPS
                    r_k = smallp.tile([128, DIM], f32, tag=f"rk{k}")
                    nc.vector.tensor_add(
                        r_k[:], selr[:, base_j + 0, 0:DIM],
                        selr[:, base_j + 1, DIM:2 * DIM])
                    nc.vector.tensor_add(
                        r_k[:], r_k[:], selr[:, base_j + 2, 2 * DIM:3 * DIM])
                    nc.vector.tensor_add(
                        r_k[:], r_k[:], selr[:, base_j + 3, 3 * DIM:4 * DIM])
                    n_k = smallp.tile([128, DIM], f32, tag=f"nk{k}")
                    nc.vector.tensor_scalar_mul(n_k[:], r_k[:], oms_ap)
                    nc.vector.tensor_add(n_k[:], n_k[:], psum_seed[:])
                    n_sums.append(n_k)

                tmp2 = smallp.tile([128, DIM], f32, tag="tmp2")

                def dotp(out_ap, a, b):
                    nc.vector.tensor_tensor(out=tmp2[:], in0=a, in1=b,
                                            op=Alu.mult)
                    nc.vector.tensor_reduce(out=out_ap, in_=tmp2[:],
                                            axis=mybir.AxisListType.X,
                                            op=Alu.add)

                pos_s = smallp.tile([128, 1], f32, tag="poss")   # raw (x16)
                dotp(pos_s[:], u_sum[:], p_sum[:])
                neg_s = []
                for k in range(K):
                    ns = smallp.tile([128, 1], f32, tag=f"negs{k}")
                    dotp(ns[:], u_sum[:], n_sums[k][:])
                    neg_s.append(ns)

                sq = smallp.tile([128, 4], f32, tag="sq")        # raw (x16)
                dotp(sq[:, 0:1], u_sum[:], u_sum[:])
                dotp(sq[:, 1:2], p_sum[:], p_sum[:])
                dotp(sq[:, 2:3], n_sums[0][:], n_sums[0][:])
                dotp(sq[:, 3:4], n_sums[1][:], n_sums[1][:])
                sq_tot = smallp.tile([128, 1], f32, tag="sqtot")
                nc.vector.tensor_add(sq_tot[:], sq[:, 0:1], sq[:, 1:2])
                nc.vector.tensor_add(sq_tot[:], sq_tot[:], sq[:, 2:3])
                nc.vector.tensor_add(sq_tot[:], sq_tot[:], sq[:, 3:4])
                nc.vector.tensor_scalar_mul(sq_tot[:], sq_tot[:], Ssc)

                negpos = smallp.tile([128, 1], f32, tag="negpos")
                nc.vector.tensor_scalar_mul(negpos[:], pos_s[:], -Ssc)
                e01 = smallp.tile([128, 2], f32, tag="e01")
                for k in range(K):
                    nc.scalar.activation(out=e01[:, k:k + 1],
                                         in_=neg_s[k][:], func=Act.Exp,
                                         bias=negpos[:], scale=Ssc)
                esum = smallp.tile([128, 1], f32, tag="esum")
                nc.vector.tensor_add(esum[:], e01[:, 0:1], e01[:, 1:2])
                mf = smallp.tile([128, 1], f32, tag="mf")
                nc.scalar.activation(out=mf[:], in_=esum[:], func=Act.Ln,
                                     bias=1.0, scale=1.0)

                nc.vector.tensor_add(part_t[:, 0:1], part_t[:, 0:1], mf[:])
                nc.vector.tensor_add(part_t[:, 1:2], part_t[:, 1:2],
                                     sq_tot[:])

            nc.sync.dma_start(part, part_t[:])
    nc.compile()
    return nc


def _build_runner(nc):
    import jax
    from jax.sharding import Mesh, PartitionSpec
    from jax.experimental.shard_map import shard_map
    from concourse import mybir
    from concourse.bass2jax import (install_neuronx_cc_hook,
                                    partition_id_tensor, _bass_exec_p)

    install_neuronx_cc_hook()
    partition_name = (nc.partition_id_tensor.name
                      if nc.partition_id_tensor else None)
    REPLICATED = {"user_tab", "item_tab", "item16"}

    in_names, out_names, out_avals, zero_outs = [], [], [], []
    for alloc in nc.m.functions[0].allocations:
        if not isinstance(alloc, mybir.MemoryLocationSet):
            continue
        name = alloc.memorylocations[0].name
        if alloc.kind == "ExternalInput":
            if name != partition_name:
                in_names.append(name)
        elif alloc.kind == "ExternalOutput":
            out_names.append(name)
            shape = tuple(alloc.tensor_shape)
            dtype = mybir.dt.np(alloc.dtype)
            out_avals.append(jax.core.ShapedArray(shape, dtype))
            zero_outs.append(np.zeros(shape, dtype))
    n_outs = len(out_avals)
    all_in_names = list(in_names) + list(out_names)
    if partition_name is not None:
        all_in_names.append(partition_name)

    def _body(*args):
        operands = list(args)
        if partition_name is not None:
            operands.append(partition_id_tensor())
        outs = _bass_exec_p.bind(
            *operands, out_avals=tuple(out_avals),
            in_names=tuple(all_in_names), out_names=tuple(out_names),
            lowering_input_output_aliases=(), sim_require_finite=True,
            sim_require_nnan=True, nc=nc)
        return tuple(outs)

    devices = jax.devices()[:NCORES]
    mesh = Mesh(np.asarray(devices), ("core",))
    spec_of = [
        PartitionSpec() if name in REPLICATED else PartitionSpec("core")
        for name in in_names
    ]
    in_specs = tuple(spec_of) + (PartitionSpec("core"),) * n_outs
    out_specs = (PartitionSpec("core"),) * n_outs
    sharded = jax.jit(
        shard_map(_body, mesh=mesh, in_specs=in_specs, out_specs=out_specs,
                  check_rep=False),
        keep_unused=True)
    shard_s = jax.sharding.NamedSharding(mesh, PartitionSpec("core"))
    repl_s = jax.sharding.NamedSharding(mesh, PartitionSpec())

    def run(per_core_maps, replicated_map):
        args = []
        for name in in_names:
            if name in REPLICATED:
                args.append(jax.device_put(replicated_map[name], repl_s))
            else:
                args.append(jax.device_put(
                    np.concatenate([m[name] for m in per_core_maps], axis=0),
                    shard_s))
        for z in zero_outs:
            args.append(jax.device_put(
                np.zeros((NCORES * z.shape[0], *z.shape[1:]), z.dtype),
                shard_s))
        outs = sharded(*args)
        return [
            {name: np.asarray(outs[i]).reshape(NCORES, *out_avals[i].shape)[c]
             for i, name in enumerate(out_names)}
            for c in range(NCORES)
        ]

    return run


def _get_runner(slots):
    key = (tuple(slots[0]), tuple(slots[1]))
    if _CACHE.get("key") != key:
        nc = _build_bass(slots)
        _CACHE["key"] = key
        _CACHE["nc"] = nc
        _CACHE["run"] = _build_runner(nc)
    return _CACHE["run"]


def combine(results):
    mf_sum = 0.0
    sq_sum = 0.0
    for r in results:
        mf_sum += float(r["part"][:, 0].astype(np.float64).sum())
        sq_sum += float(r["part"][:, 1].astype(np.float64).sum())
    mf_loss = np.float32(mf_sum / BATCH)
    emb_loss = np.float32(DECAY * sq_sum / 2.0 / BATCH)
    loss = np.float32(mf_loss + emb_loss)
    return loss, mf_loss, emb_loss


def kernel(user_gcn_emb, item_gcn_emb, seed_embed, user, pos_item, neg_item):
    per_core, replicated, slots = make_in_maps(
        user_gcn_emb, item_gcn_emb, seed_embed, user, pos_item, neg_item)
    run = _get_runner(slots)
    results = run(per_core, replicated)
    return combine(results)
